# revision 64
# baseline (speedup 1.0000x reference)
"""Trainium2 Bass kernel for nn_Attention_4501125726440 (sparse_attention).

Full attention layer: QKV projections, per-head-dim RMSNorm on Q/K, full-head
RoPE, causal attention with sink-augmented softmax, output projection.

Sharding: 8 cores = (batch b in {0,1}) x (head-group hg in {0..3}, 4 heads
each).  Each core computes its batch's 4 heads end-to-end plus the partial
output projection through the matching 256 rows of wo^T; the host sums the 4
partials per batch (row-parallel tensor parallelism).

v3 design notes (per core; B=2, S=2048, D=1024, H=16, HD=64; 4 heads/core):
  - x arrives host-transposed as xT [D, S]; Q/K are computed feature-major
    ([256 feats on partitions, S free]) so attention score matmuls need no
    transposes; V is computed sequence-major from lhsT = xT tiles.
  - Q/K phase is a 2-stage software pipeline over 16 [128,512] chunks:
    stage1 = {square, raw-cast (scalar); sum-sq reduce (PE); abs-rsqrt
    (scalar, fused 1/sqrt(ms/HD+eps))}, stage2 = {norm-weight broadcast
    matmul (PE); raw*sin, raw*cos (vector); rotate-half permutation matmul
    (PE); (rot + qc) then *bsc-psum (vector)}.  The RMS scale commutes with
    the rotation so it is applied once at the end; PE never waits on the
    scalar/vector chain of the in-flight chunk.
  - V-projection q-tiles are interleaved into the attention schedule (PE has
    slack there while ScalarE runs exp); V lands in one [128, kb, 4, 65]
    tile whose 65th column of ones makes the PV matmul's row 64 accumulate
    the softmax denominator.
  - Attention: scores K^T Q land feature-major so exp applies blockwise on
    ScalarE (both heads of a chunk in one strided activation); causal
    diagonal masked post-exp on vector; PV accumulates [65, 2, 512] in one
    PSUM tile; denominator row + exp(sink) -> reciprocal -> gpsimd
    partition-broadcast -> normalize.  No max-subtraction needed
    (post-RMSNorm |scores| <= ~8).
  - y is emitted as float16 (halves the HBM write; fp16 partials are
    accurate to ~1e-3 here) and summed across head-group cores on the host.
  - Warm-up: junk matmuls keep the PE HAM clock gate at 2.4 GHz through the
    input DMA ramp; a dummy partition_broadcast forces the GpSimd custom-op
    library load (~7us) off the critical path; dummy activations preload
    the ACT tables.

Matmul inputs are bf16 (cast on host); accumulation is fp32 in PSUM.
"""

import sys

import ml_dtypes
import numpy as np

_REPO = "/opt/trn_rl_repo"
if _REPO not in sys.path:
    sys.path.insert(0, _REPO)

import concourse.bacc as bacc  # noqa: E402
import concourse.mybir as mybir  # noqa: E402
import concourse.tile as tile  # noqa: E402
from concourse.bass_utils import run_bass_kernel_spmd  # noqa: E402

B, S, D = 2, 2048, 1024
H = 16
HD = 64
HEADS_PER_CORE = 4
FEATS = HEADS_PER_CORE * HD  # 256
EPS = 1e-6
ROPE_BASE = 10000.0
N_CORES = 8

F32 = mybir.dt.float32
F16 = mybir.dt.float16
BF16 = mybir.dt.bfloat16
BF16_NP = ml_dtypes.bfloat16

DCH = D // 128      # 8 contraction chunks for projections
FCH = FEATS // 128  # 2 feature chunks (2 heads each)
SQ = 512            # chunk width in projections / q-tile width in attention
NSQ = S // SQ       # 4
NKB = S // 128      # 16 key blocks
NQT = S // 128      # 16 128-row q tiles

EXP = mybir.ActivationFunctionType.Exp
# 1/sqrt(|x|): input (mean square + eps) is >= 0, so this is rsqrt.
ARSQRT = mybir.ActivationFunctionType.Abs_reciprocal_sqrt
SQUARE = mybir.ActivationFunctionType.Square

# Apply the causal diagonal mask by accumulating a -30 upper-triangle bias
# into the scores PSUM with an identity matmul (True), vs multiplying the
# exp output by a 0/1 mask on the vector engine (False).
MASK_VIA_PE = False


def build_program():
    nc = bacc.Bacc("TRN2", target_bir_lowering=False, debug=False)

    xT = nc.dram_tensor("xT", [D, S], BF16, kind="ExternalInput").ap()
    # weights pre-shuffled on host to partition-major [128, o*f] so each
    # load is one full-line-rate DMA
    wqS = nc.dram_tensor("wqS", [128, DCH * FEATS], BF16, kind="ExternalInput").ap()
    wkS = nc.dram_tensor("wkS", [128, DCH * FEATS], BF16, kind="ExternalInput").ap()
    wvS = nc.dram_tensor("wvS", [128, DCH * FEATS], BF16, kind="ExternalInput").ap()
    woS = nc.dram_tensor("woS", [128, FCH * D], BF16, kind="ExternalInput").ap()
    cosT = nc.dram_tensor("cosT", [128, S], BF16, kind="ExternalInput").ap()
    sinT = nc.dram_tensor("sinT", [128, S], BF16, kind="ExternalInput").ap()
    trimask2 = nc.dram_tensor("trimask2", [128, 2 * 128], BF16,
                              kind="ExternalInput").ap()
    identm = nc.dram_tensor("identm", [128, 128], BF16, kind="ExternalInput").ap()
    maskbias = nc.dram_tensor("maskbias", [128, 128], BF16,
                              kind="ExternalInput").ap()
    bd1 = nc.dram_tensor("bd1", [128, 2], BF16, kind="ExternalInput").ap()
    qbd = nc.dram_tensor("qbd", [2, 128], BF16, kind="ExternalInput").ap()
    kbd = nc.dram_tensor("kbd", [2, 128], BF16, kind="ExternalInput").ap()
    rotm = nc.dram_tensor("rotm", [128, 128], BF16, kind="ExternalInput").ap()
    sinkexp = nc.dram_tensor("sinkexp", [1, HEADS_PER_CORE], F32,
                             kind="ExternalInput").ap()
    y = nc.dram_tensor("y", [S, D], F16, kind="ExternalOutput").ap()

    xT3 = xT.rearrange("(o p) s -> p o s", p=128)      # [128, 8, S]
    wqT3 = wqS.rearrange("p (o f) -> p o f", o=DCH)    # [128, 8, 256]
    wkT3 = wkS.rearrange("p (o f) -> p o f", o=DCH)
    wvT3 = wvS.rearrange("p (o f) -> p o f", o=DCH)
    woT3 = woS.rearrange("p (o d) -> p o d", o=FCH)    # [128, 2, 1024]

    with tile.TileContext(nc) as tc, nc.allow_low_precision(reason="bf16 matmul pipeline"):
        with (
            tc.tile_pool(name="persist", bufs=1) as persist,
            tc.tile_pool(name="consts", bufs=1) as consts,
        ):
            # Persistent SBUF tensors
            q_sb = persist.tile([128, FCH, S], BF16, tag="q_sb")
            k_sb = persist.tile([128, FCH, S], BF16, tag="k_sb")
            # V per head along dim2, each with a 65th ones column
            v_sb = persist.tile([128, NKB, HEADS_PER_CORE, HD + 1], BF16,
                                tag="v_sb")
            ot_sb = persist.tile([128, FCH, S], BF16, tag="ot_sb")
            vf_sb = persist.tile([128, FCH, S], BF16, tag="vf_sb")
            xt_sb = persist.tile([128, DCH, S], BF16, tag="xt_sb")

            cos_sb = consts.tile([128, S], BF16, tag="cos_sb")
            sin_sb = consts.tile([128, S], BF16, tag="sin_sb")
            mask_sb = consts.tile([128, 2, 128], BF16, tag="mask_sb")
            bd1_sb = consts.tile([128, 2], BF16, tag="bd1_sb")
            qbd_sb = consts.tile([2, 128], BF16, tag="qbd_sb")
            kbd_sb = consts.tile([2, 128], BF16, tag="kbd_sb")
            rotm_sb = consts.tile([128, 128], BF16, tag="rotm_sb")
            sinkf_sb = consts.tile([1, HEADS_PER_CORE], F32, tag="sinkf_sb")
            eps_sb = consts.tile([128, 1], F32, tag="eps_sb")
            ident_sb = consts.tile([128, 128], BF16, tag="ident_sb")
            mbias_sb = consts.tile([128, 128], BF16, tag="mbias_sb")
            if MASK_VIA_PE:
                nc.gpsimd.dma_start(ident_sb[:], identm)
                nc.gpsimd.dma_start(mbias_sb[:], maskbias)

            # weights
            wq_sb = persist.tile([128, DCH, FEATS], BF16, tag="wq_sb")
            wk_sb = persist.tile([128, DCH, FEATS], BF16, tag="wk_sb")
            wv_sb = persist.tile([128, DCH, FEATS], BF16, tag="wv_sb")
            wo_sb = persist.tile([128, FCH, D], BF16, tag="wo_sb")

            # --- input DMAs ------------------------------------------------
            # x in 16 [128,1024] halves (2KB partition lines = full DMA line
            # rate); the first-need set (all d of half 0 -> chunks s0+s1)
            # spread over the three DMA rings.  Weights are single
            # partition-major full-rate DMAs.
            def xh(eng, d, h):
                eng.dma_start(xt_sb[:, d, h * 1024:(h + 1) * 1024],
                              xT3[:, d, h * 1024:(h + 1) * 1024])

            nc.sync.dma_start(wq_sb[:], wqT3)
            xh(nc.sync, 0, 0), xh(nc.sync, 1, 0), xh(nc.sync, 2, 0)
            nc.sync.dma_start(bd1_sb[:], bd1)
            nc.sync.dma_start(qbd_sb[:], qbd)
            nc.sync.dma_start(kbd_sb[:], kbd)
            nc.sync.dma_start(rotm_sb[:], rotm)
            xh(nc.sync, 0, 1), xh(nc.sync, 1, 1), xh(nc.sync, 2, 1)

            xh(nc.scalar, 3, 0), xh(nc.scalar, 4, 0), xh(nc.scalar, 5, 0)
            nc.scalar.dma_start(cos_sb[:, 0:1024], cosT[:, 0:1024])
            nc.scalar.dma_start(sin_sb[:, 0:1024], sinT[:, 0:1024])
            xh(nc.scalar, 3, 1), xh(nc.scalar, 4, 1), xh(nc.scalar, 5, 1)
            nc.scalar.dma_start(cos_sb[:, 1024:S], cosT[:, 1024:S])
            nc.scalar.dma_start(sin_sb[:, 1024:S], sinT[:, 1024:S])

            # Warm-up, interleaved with the remaining loads: junk matmuls
            # open the HAM clock gate during the DMA ramp, a dummy
            # partition_broadcast forces the ~7us gpsimd LOAD_LIB early, and
            # dummy activations preload the ACT tables.  All tiles live in
            # the long-lived consts pool: a short-lived pool's exit barrier
            # would handcuff the vector queue to the slow gpsimd chain.
            wz = consts.tile([128, 256], BF16, tag="wz")
            nc.vector.memset(wz[:], 0.0)
            nc.vector.memset(eps_sb[:], EPS)
            nc.vector.memset(v_sb[:, :, :, HD:HD + 1], 1.0)
            wact = consts.tile([2, 16], F32, tag="wact")
            nc.vector.memset(wact[:], 1.0)

            xh(nc.gpsimd, 6, 0), xh(nc.gpsimd, 7, 0)
            nc.gpsimd.dma_start(wk_sb[:], wkT3)
            xh(nc.gpsimd, 6, 1), xh(nc.gpsimd, 7, 1)
            wbc = consts.tile([2, 16], F32, tag="wbc")
            nc.gpsimd.partition_broadcast(wbc[:], wact[0:1, :])
            nc.gpsimd.dma_start(wv_sb[:], wvT3)
            nc.gpsimd.dma_start(wo_sb[:], woT3)
            nc.gpsimd.dma_start(
                mask_sb[:], trimask2.rearrange("p (a b) -> p a b", a=2))
            nc.gpsimd.dma_start(sinkf_sb[:], sinkexp)

            wacto = consts.tile([2, 16], F32, tag="wacto")
            nc.scalar.activation(wacto[:], wact[:], EXP)
            nc.scalar.activation(wacto[:], wact[:], ARSQRT)
            nc.scalar.activation(wacto[:], wact[:], SQUARE)

            # ---------------- Phase A: Q/K projections + norm + rope ------
            # Two-stage software pipeline so the PE stream (proj of chunk c,
            # then sum-sq of c-1, then bsc/rot of c-2) never waits on the
            # scalar/vector chain of the chunk in flight.
            with (
                tc.tile_pool(name="pa_w", bufs=3) as paw,
                tc.tile_pool(name="pa_ps", bufs=2, space="PSUM") as paps,
                tc.tile_pool(name="pa_ss", bufs=2, space="PSUM") as passs,
                tc.tile_pool(name="pa_bsc", bufs=2, space="PSUM") as pabsc,
                tc.tile_pool(name="pa_rot", bufs=2, space="PSUM") as parot,
            ):
                # s-major so 4 consecutive chunks share one x quarter-set
                chunks = [(p, f, s) for s in range(NSQ)
                          for f in range(FCH) for p in ("q", "k")]

                # HAM warm-up junk matmuls during the input DMA ramp
                wps = paps.tile([128, SQ], F32, tag="qk_ps", name="wps")
                for i in range(36):
                    nc.tensor.matmul(wps[:, 0:256], wz[:, 0:128], wz[:],
                                     start=True, stop=True)

                def stage1(st):
                    p, f, s, ps = st
                    sq = paw.tile([128, SQ], BF16, tag="sq")
                    nc.scalar.square(sq[:], ps[:])
                    raw = paw.tile([128, SQ], BF16, tag="raw")
                    nc.scalar.copy(raw[:], ps[:])
                    ss_ps = passs.tile([128, SQ], F32, tag="ss_ps",
                                       name="ss_ps")[0:2]
                    nc.tensor.matmul(ss_ps[:], bd1_sb[:], sq[:],
                                     start=True, stop=True)
                    rinv = paw.tile([2, SQ], BF16, tag="rinv")
                    nc.scalar.activation(rinv[:], ss_ps[:], ARSQRT,
                                         bias=eps_sb[0:2, :], scale=1.0 / HD)
                    st.append(raw)
                    st.append(rinv)

                def stage2(st):
                    p, f, s, ps, raw, rinv = st
                    bdw_sb = qbd_sb if p == "q" else kbd_sb
                    dst_sb = q_sb if p == "q" else k_sb
                    cols = slice(s * SQ, (s + 1) * SQ)
                    bsc_ps = pabsc.tile([128, SQ], F32, tag="bsc_ps")
                    nc.tensor.matmul(bsc_ps[:], bdw_sb[:], rinv[:],
                                     start=True, stop=True)
                    qs = paw.tile([128, SQ], BF16, tag="qs")
                    nc.vector.tensor_mul(qs[:], raw[:], sin_sb[:, cols])
                    rot_ps = parot.tile([128, SQ], F32, tag="rot_ps")
                    nc.tensor.matmul(rot_ps[:], rotm_sb[:], qs[:],
                                     start=True, stop=True)
                    qc = paw.tile([128, SQ], BF16, tag="qc")
                    nc.vector.tensor_mul(qc[:], raw[:], cos_sb[:, cols])
                    qr = paw.tile([128, SQ], BF16, tag="qr")
                    nc.vector.tensor_add(qr[:], rot_ps[:], qc[:])
                    nc.vector.tensor_mul(dst_sb[:, f, cols], qr[:], bsc_ps[:])

                states = {}
                n = len(chunks)
                for c in range(n + 2):
                    if c < n:
                        p, f, s = chunks[c]
                        w_sb = wq_sb if p == "q" else wk_sb
                        cols = slice(s * SQ, (s + 1) * SQ)
                        ps = paps.tile([128, SQ], F32, tag="qk_ps")
                        for d in range(DCH):
                            nc.tensor.matmul(
                                ps[:],
                                w_sb[:, d, f * 128:(f + 1) * 128],
                                xt_sb[:, d, cols],
                                start=(d == 0),
                                stop=(d == DCH - 1),
                            )
                        if c < 4:
                            # early chunks are DMA-arrival-paced; pad the PE
                            # stream so the HAM clock gate stays open
                            for i in range(6):
                                nc.tensor.matmul(wps[:, 0:256], wz[:, 0:128],
                                                 wz[:], start=True, stop=True)
                        states[c] = [p, f, s, ps]
                    if c - 1 >= 0 and c - 1 < n:
                        stage1(states[c - 1])
                    if c - 2 >= 0:
                        stage2(states.pop(c - 2))
                # reload the EXP table set (evicted by SQUARE/ARSQRT) during
                # the phase transition instead of before the first real exp
                nc.scalar.activation(wacto[:], wact[:], EXP)

            # ------------- Phase C: attention + V-proj + output proj ------
            with (
                tc.tile_pool(name="pc_p", bufs=6) as pcp,
                tc.tile_pool(name="pc_w", bufs=2) as pcw,
                tc.tile_pool(name="pc_y", bufs=3) as pcy,
                tc.tile_pool(name="pc_s_ps", bufs=2, space="PSUM") as pcsps,
                tc.tile_pool(name="pc_o_ps", bufs=1, space="PSUM") as pcops,
                tc.tile_pool(name="pc_y_ps", bufs=2, space="PSUM") as pcyps,
            ):
                def v_proj(qt):
                    # V projection for one q-tile, borrowing the y PSUM pool
                    v_ps = pcyps.tile([128, SQ], F32, tag="y_ps", name="v_ps")
                    for d in range(DCH):
                        nc.tensor.matmul(
                            v_ps[:, 0:FEATS],
                            xt_sb[:, d, qt * 128:(qt + 1) * 128],
                            wv_sb[:, d, :],
                            start=(d == 0),
                            stop=(d == DCH - 1),
                        )
                    nc.vector.tensor_copy(
                        v_sb[:, qt, :, 0:HD],
                        v_ps[:, 0:FEATS].rearrange("p (h e) -> p h e",
                                                   h=HEADS_PER_CORE))

                vpend, ypend = [], []

                def mk_v_halves(qt):
                    # V projection split into two 4-matmul halves so a
                    # filler never exceeds the exp pipeline's ~1-iter
                    # run-ahead
                    st = {}

                    def h0():
                        st["ps"] = pcyps.tile([128, SQ], F32, tag="y_ps",
                                              name="v_ps")
                        for d in range(4):
                            nc.tensor.matmul(
                                st["ps"][:, 0:FEATS],
                                xt_sb[:, d, qt * 128:(qt + 1) * 128],
                                wv_sb[:, d, :],
                                start=(d == 0), stop=False)

                    def h1():
                        for d in range(4, DCH):
                            nc.tensor.matmul(
                                st["ps"][:, 0:FEATS],
                                xt_sb[:, d, qt * 128:(qt + 1) * 128],
                                wv_sb[:, d, :],
                                start=False, stop=(d == DCH - 1))
                        nc.vector.tensor_copy(
                            v_sb[:, qt, :, 0:HD],
                            st["ps"][:, 0:FEATS].rearrange(
                                "p (h e) -> p h e", h=HEADS_PER_CORE))
                    return [h0, h1]

                def mk_out_halves(qt):
                    st = {}

                    def half(n):
                        if n == 0:
                            st["y"] = pcy.tile([128, D], F16, tag="y_sb",
                                               name="y_sb")
                        y_ps = pcyps.tile([128, SQ], F32, tag="y_ps",
                                          name="y_ps")
                        for c in range(FCH):
                            nc.tensor.matmul(
                                y_ps[:],
                                ot_sb[:, c, qt * 128:(qt + 1) * 128],
                                wo_sb[:, c, n * SQ:(n + 1) * SQ],
                                start=(c == 0), stop=(c == FCH - 1))
                        nc.vector.tensor_copy(
                            st["y"][:, n * SQ:(n + 1) * SQ], y_ps[:])
                        if n == 1:
                            nc.sync.dma_start(y[qt * 128:(qt + 1) * 128, :],
                                              st["y"][:])
                    return [lambda: half(0), lambda: half(1)]

                def out_proj_qt(qt):
                    y_sb = pcy.tile([128, D], F16, tag="y_sb", name="y_sb")
                    for n in range(D // SQ):
                        y_ps = pcyps.tile([128, SQ], F32, tag="y_ps",
                                          name="y_ps")
                        for c in range(FCH):
                            nc.tensor.matmul(
                                y_ps[:],
                                ot_sb[:, c, qt * 128:(qt + 1) * 128],
                                wo_sb[:, c, n * SQ:(n + 1) * SQ],
                                start=(c == 0),
                                stop=(c == FCH - 1),
                            )
                        nc.vector.tensor_copy(y_sb[:, n * SQ:(n + 1) * SQ],
                                              y_ps[:])
                    nc.sync.dma_start(y[qt * 128:(qt + 1) * 128, :], y_sb[:])

                def attention_tf(t, f):
                    nkb = (t + 1) * (SQ // 128)
                    ot_ps = None
                    for i, kb in enumerate(range(nkb)):
                        j = kb - (t * (SQ // 128))
                        qlo = max(j, 0) * 128
                        diag = j >= 0
                        sp = pcsps.tile([128, 2, SQ], F32, tag="s_ps")
                        for hh in range(2):
                            plo = hh * HD
                            nc.tensor.matmul(
                                sp[:, hh, qlo:],
                                k_sb[plo:plo + HD, f, kb * 128:(kb + 1) * 128],
                                q_sb[plo:plo + HD, f, t * SQ + qlo:(t + 1) * SQ],
                                start=True, stop=True,
                            )
                        p_sb = pcp.tile([128, 2, SQ], BF16, tag="p_sb")
                        nc.scalar.activation(p_sb[:, :, qlo:],
                                             sp[:, :, qlo:], EXP)
                        if diag:
                            nc.vector.tensor_mul(
                                p_sb[:, :, qlo:qlo + 128],
                                p_sb[:, :, qlo:qlo + 128],
                                mask_sb[:])
                        if i == 0:
                            # allocated late: the single-buffered pool reuses
                            # the previous tile's banks only after its
                            # denominator reads drained
                            ot_ps = pcops.tile([HD + 1, 2, SQ], F32,
                                               tag="ot_ps", name="ot_ps")
                        for hh in range(2):
                            h = 2 * f + hh
                            nc.tensor.matmul(
                                ot_ps[:, hh, qlo:],
                                v_sb[:, kb, h, :],
                                p_sb[:, hh, qlo:],
                                start=(i == 0),
                                stop=(i == nkb - 1),
                            )
                        # half-size V/out-proj fillers absorb the PE idle
                        # that the exp-paced pipeline leaves each iteration
                        if i % 2 == 1:
                            if vpend:
                                vpend.pop(0)()
                            elif i % 4 == 3 and ypend:
                                ypend.pop(0)()
                    # denominators: row HD of ot_ps + exp(sink), batched hh
                    draw = pcw.tile([1, 2, SQ], F32, tag="draw")
                    nc.vector.tensor_scalar_add(
                        draw[:, 0, :], ot_ps[HD:HD + 1, 0, :],
                        sinkf_sb[0:1, 2 * f:2 * f + 1])
                    nc.vector.tensor_scalar_add(
                        draw[:, 1, :], ot_ps[HD:HD + 1, 1, :],
                        sinkf_sb[0:1, 2 * f + 1:2 * f + 2])
                    den = pcw.tile([1, 2, SQ], F32, tag="den")
                    nc.vector.reciprocal_approx_fast(den[:], draw[:])
                    last = (t == NSQ - 1 and f == 1)
                    if not last:
                        # drain ot_ps via the boundary-idle scalar engine so
                        # the single-buffered pool frees before the next
                        # tile's first PV matmul
                        ot_u = pcw.tile([HD, 2, SQ], BF16, tag="ot_u")
                        nc.scalar.copy(ot_u[:], ot_ps[0:HD, :, :])
                    for hh in range(2):
                        plo = hh * HD
                        bc_sb = pcw.tile([HD, SQ], F32, tag=f"bc_sb{hh}")
                        nc.gpsimd.partition_broadcast(bc_sb[:], den[:, hh, :])
                        nc.vector.tensor_mul(
                            ot_sb[plo:plo + HD, f, t * SQ:(t + 1) * SQ],
                            ot_ps[0:HD, hh, :] if last else ot_u[:, hh, :],
                            bc_sb[:])
                    if f == 1:
                        for qi in range(SQ // 128):
                            qt = t * (SQ // 128) + qi
                            ypend.extend(mk_out_halves(qt))

                # V q-tiles feed attention tile t's key blocks (<= 4t+3),
                # so they are threaded through the schedule just ahead of
                # need; out_proj q-tiles trail as PE fillers inside later
                # attention kb loops.
                junk_ps0 = pcsps.tile([128, 2, SQ], F32, tag="s_ps",
                                      name="junk_ps0")
                for i in range(8):
                    nc.tensor.matmul(junk_ps0[:, 0, 0:256], wz[:, 0:128],
                                     wz[:], start=True, stop=True)
                def flushv():
                    while vpend:
                        vpend.pop(0)()

                for qt in range(4):
                    v_proj(qt)
                vpend.extend(mk_v_halves(4) + mk_v_halves(5))
                attention_tf(0, 0)
                flushv()
                vpend.extend(mk_v_halves(6) + mk_v_halves(7))
                attention_tf(0, 1)
                flushv()
                vpend.extend(mk_v_halves(8) + mk_v_halves(9))
                attention_tf(1, 0)
                flushv()
                vpend.extend(mk_v_halves(10) + mk_v_halves(11))
                attention_tf(1, 1)
                flushv()
                while len(ypend) > 4:
                    ypend.pop(0)()
                vpend.extend(mk_v_halves(12) + mk_v_halves(13))
                attention_tf(2, 0)
                flushv()
                vpend.extend(mk_v_halves(14) + mk_v_halves(15))
                attention_tf(2, 1)
                flushv()
                while len(ypend) > 4:
                    ypend.pop(0)()
                attention_tf(3, 0)
                attention_tf(3, 1)
                junk_ps = pcsps.tile([128, 2, SQ], F32, tag="s_ps",
                                     name="junk_ps")

                def junk(n):
                    # keep the HAM clock gate open across dependency waits
                    for i in range(n):
                        nc.tensor.matmul(junk_ps[:, 0, 0:256], wz[:, 0:128],
                                         wz[:], start=True, stop=True)
                junk(6)
                while ypend:
                    ypend.pop(0)()
                    junk(2)

    nc.compile()
    return nc


_NC_CACHE = None


def _get_program():
    global _NC_CACHE
    if _NC_CACHE is None:
        _NC_CACHE = build_program()
    return _NC_CACHE


def _b(x):
    return np.ascontiguousarray(np.asarray(x, dtype=np.float32)).astype(BF16_NP)


def _shuf(wT, n):
    """[O*128, n] -> partition-major [128, O*n] (one full-line-rate DMA)."""
    o = wT.shape[0] // 128
    return np.ascontiguousarray(
        wT.reshape(o, 128, n).transpose(1, 0, 2).reshape(128, o * n))


def _host_inputs(x, wq, wk, wv, wo, q_norm_w, k_norm_w, sink_logit):
    """Build the 8 per-core input maps."""
    x = np.asarray(x, dtype=np.float32)
    wq = np.asarray(wq, dtype=np.float32)
    wk = np.asarray(wk, dtype=np.float32)
    wv = np.asarray(wv, dtype=np.float32)
    wo = np.asarray(wo, dtype=np.float32)
    q_norm_w = np.asarray(q_norm_w, dtype=np.float32)
    k_norm_w = np.asarray(k_norm_w, dtype=np.float32)
    sink_logit = np.asarray(sink_logit, dtype=np.float32)

    # rope tables, feature-major, duplicated across the two heads per chunk
    inv_freq = 1.0 / (ROPE_BASE ** (np.arange(0, HD, 2, dtype=np.float32) / HD))
    tpos = np.arange(S, dtype=np.float32)
    freqs = tpos[:, None] * inv_freq[None, :]           # [S, 32]
    emb = np.concatenate([freqs, freqs], axis=-1)       # [S, 64]
    cosT = _b(np.tile(np.cos(emb).T, (2, 1)))           # [128, S]
    sinT = _b(np.tile(np.sin(emb).T, (2, 1)))

    # triangular causal mask for the diagonal 128-wide band, doubled for
    # the two heads of a chunk
    kk = np.arange(128)[:, None]
    qq = np.arange(128)[None, :]
    tm = (kk <= qq).astype(np.float32)                  # [128, 128]
    trimask2 = _b(np.concatenate([tm, tm], axis=1))     # [128, 256]

    bd1 = np.zeros((128, 2), dtype=np.float32)
    bd1[0:64, 0] = 1.0
    bd1[64:128, 1] = 1.0
    bd1 = _b(bd1)

    scale_half = float(HD) ** -0.25  # sqrt of softmax scale, folded into q & k
    qbd = np.zeros((2, 128), dtype=np.float32)
    kbd = np.zeros((2, 128), dtype=np.float32)
    for m in range(2):
        qbd[m, m * 64:(m + 1) * 64] = q_norm_w * scale_half
        kbd[m, m * 64:(m + 1) * 64] = k_norm_w * scale_half
    qbd = _b(qbd)
    kbd = _b(kbd)

    rotm = np.zeros((128, 128), dtype=np.float32)
    for base in (0, 64):
        for m in range(32):
            rotm[base + m + 32, base + m] = -1.0
            rotm[base + m, base + m + 32] = 1.0
    rotm = _b(rotm)

    identm = _b(np.eye(128, dtype=np.float32))
    maskbias = _b(np.where(kk > qq, -30.0, 0.0).astype(np.float32))

    in_maps = []
    xT_b = [_b(x[b].T) for b in range(B)]
    for core in range(N_CORES):
        b = core // 4
        hg = core % 4
        rows = slice(hg * FEATS, (hg + 1) * FEATS)
        heads = slice(hg * HEADS_PER_CORE, (hg + 1) * HEADS_PER_CORE)
        in_maps.append({
            "xT": xT_b[b],
            "wqS": _shuf(_b(wq[rows, :].T), FEATS),
            "wkS": _shuf(_b(wk[rows, :].T), FEATS),
            "wvS": _shuf(_b(wv[rows, :].T), FEATS),
            "woS": _shuf(_b(wo[:, rows].T), D),
            "cosT": cosT,
            "sinT": sinT,
            "trimask2": trimask2,
            "identm": identm,
            "maskbias": maskbias,
            "bd1": bd1,
            "qbd": qbd,
            "kbd": kbd,
            "rotm": rotm,
            "sinkexp": np.exp(sink_logit[heads]).astype(np.float32).reshape(
                1, HEADS_PER_CORE),
        })
    return in_maps


def kernel(x, wq, wk, wv, wo, q_norm_w, k_norm_w, sink_logit, _run_kwargs=None):
    nc = _get_program()
    in_maps = _host_inputs(x, wq, wk, wv, wo, q_norm_w, k_norm_w, sink_logit)
    res = run_bass_kernel_spmd(nc, in_maps, core_ids=list(range(N_CORES)),
                               **(_run_kwargs or {}))
    out = np.zeros((B, S, D), dtype=np.float32)
    for core in range(N_CORES):
        out[core // 4] += res.results[core]["y"].astype(np.float32)
    if _run_kwargs:
        kernel.last_result = res
    return out


# revision 67
# speedup vs baseline: 1.0026x; 1.0026x over previous
"""Trainium2 Bass kernel for nn_Attention_4501125726440 (sparse_attention).

Full attention layer: QKV projections, per-head-dim RMSNorm on Q/K, full-head
RoPE, causal attention with sink-augmented softmax, output projection.

Sharding: 8 cores = (batch b in {0,1}) x (head-group hg in {0..3}, 4 heads
each).  Each core computes its batch's 4 heads end-to-end plus the partial
output projection through the matching 256 rows of wo^T; the host sums the 4
partials per batch (row-parallel tensor parallelism).

v3 design notes (per core; B=2, S=2048, D=1024, H=16, HD=64; 4 heads/core):
  - x arrives host-transposed as xT [D, S]; Q/K are computed feature-major
    ([256 feats on partitions, S free]) so attention score matmuls need no
    transposes; V is computed sequence-major from lhsT = xT tiles.
  - Q/K phase is a 2-stage software pipeline over 16 [128,512] chunks:
    stage1 = {square, raw-cast (scalar); sum-sq reduce (PE); abs-rsqrt
    (scalar, fused 1/sqrt(ms/HD+eps))}, stage2 = {norm-weight broadcast
    matmul (PE); raw*sin, raw*cos (vector); rotate-half permutation matmul
    (PE); (rot + qc) then *bsc-psum (vector)}.  The RMS scale commutes with
    the rotation so it is applied once at the end; PE never waits on the
    scalar/vector chain of the in-flight chunk.
  - V-projection q-tiles are interleaved into the attention schedule (PE has
    slack there while ScalarE runs exp); V lands in one [128, kb, 4, 65]
    tile whose 65th column of ones makes the PV matmul's row 64 accumulate
    the softmax denominator.
  - Attention: scores K^T Q land feature-major so exp applies blockwise on
    ScalarE (both heads of a chunk in one strided activation); causal
    diagonal masked post-exp on vector; PV accumulates [65, 2, 512] in one
    PSUM tile; denominator row + exp(sink) -> reciprocal -> gpsimd
    partition-broadcast -> normalize.  No max-subtraction needed
    (post-RMSNorm |scores| <= ~8).
  - y is emitted as float16 (halves the HBM write; fp16 partials are
    accurate to ~1e-3 here) and summed across head-group cores on the host.
  - Warm-up: junk matmuls keep the PE HAM clock gate at 2.4 GHz through the
    input DMA ramp; a dummy partition_broadcast forces the GpSimd custom-op
    library load (~7us) off the critical path; dummy activations preload
    the ACT tables.

Matmul inputs are bf16 (cast on host); accumulation is fp32 in PSUM.
"""

import sys

import ml_dtypes
import numpy as np

_REPO = "/opt/trn_rl_repo"
if _REPO not in sys.path:
    sys.path.insert(0, _REPO)

import concourse.bacc as bacc  # noqa: E402
import concourse.mybir as mybir  # noqa: E402
import concourse.tile as tile  # noqa: E402
from concourse.bass_utils import run_bass_kernel_spmd  # noqa: E402

B, S, D = 2, 2048, 1024
H = 16
HD = 64
HEADS_PER_CORE = 4
FEATS = HEADS_PER_CORE * HD  # 256
EPS = 1e-6
ROPE_BASE = 10000.0
N_CORES = 8

F32 = mybir.dt.float32
F16 = mybir.dt.float16
BF16 = mybir.dt.bfloat16
BF16_NP = ml_dtypes.bfloat16

DCH = D // 128      # 8 contraction chunks for projections
FCH = FEATS // 128  # 2 feature chunks (2 heads each)
SQ = 512            # chunk width in projections / q-tile width in attention
NSQ = S // SQ       # 4
NKB = S // 128      # 16 key blocks
NQT = S // 128      # 16 128-row q tiles

EXP = mybir.ActivationFunctionType.Exp
# 1/sqrt(|x|): input (mean square + eps) is >= 0, so this is rsqrt.
ARSQRT = mybir.ActivationFunctionType.Abs_reciprocal_sqrt
SQUARE = mybir.ActivationFunctionType.Square

# Apply the causal diagonal mask by accumulating a -30 upper-triangle bias
# into the scores PSUM with an identity matmul (True), vs multiplying the
# exp output by a 0/1 mask on the vector engine (False).
MASK_VIA_PE = False


def build_program():
    nc = bacc.Bacc("TRN2", target_bir_lowering=False, debug=False)

    xT = nc.dram_tensor("xT", [D, S], BF16, kind="ExternalInput").ap()
    # weights pre-shuffled on host to partition-major [128, o*f] so each
    # load is one full-line-rate DMA
    wqS = nc.dram_tensor("wqS", [128, DCH * FEATS], BF16, kind="ExternalInput").ap()
    wkS = nc.dram_tensor("wkS", [128, DCH * FEATS], BF16, kind="ExternalInput").ap()
    wvS = nc.dram_tensor("wvS", [128, DCH * FEATS], BF16, kind="ExternalInput").ap()
    woS = nc.dram_tensor("woS", [128, FCH * D], BF16, kind="ExternalInput").ap()
    cosT = nc.dram_tensor("cosT", [128, S], BF16, kind="ExternalInput").ap()
    sinT = nc.dram_tensor("sinT", [128, S], BF16, kind="ExternalInput").ap()
    trimask2 = nc.dram_tensor("trimask2", [128, 2 * 128], BF16,
                              kind="ExternalInput").ap()
    identm = nc.dram_tensor("identm", [128, 128], BF16, kind="ExternalInput").ap()
    maskbias = nc.dram_tensor("maskbias", [128, 128], BF16,
                              kind="ExternalInput").ap()
    bd1 = nc.dram_tensor("bd1", [128, 2], BF16, kind="ExternalInput").ap()
    qbd = nc.dram_tensor("qbd", [2, 128], BF16, kind="ExternalInput").ap()
    kbd = nc.dram_tensor("kbd", [2, 128], BF16, kind="ExternalInput").ap()
    rotm = nc.dram_tensor("rotm", [128, 128], BF16, kind="ExternalInput").ap()
    sinkexp = nc.dram_tensor("sinkexp", [1, HEADS_PER_CORE], F32,
                             kind="ExternalInput").ap()
    y = nc.dram_tensor("y", [S, D], F16, kind="ExternalOutput").ap()

    xT3 = xT.rearrange("(o p) s -> p o s", p=128)      # [128, 8, S]
    wqT3 = wqS.rearrange("p (o f) -> p o f", o=DCH)    # [128, 8, 256]
    wkT3 = wkS.rearrange("p (o f) -> p o f", o=DCH)
    wvT3 = wvS.rearrange("p (o f) -> p o f", o=DCH)
    woT3 = woS.rearrange("p (o d) -> p o d", o=FCH)    # [128, 2, 1024]

    with tile.TileContext(nc) as tc, nc.allow_low_precision(reason="bf16 matmul pipeline"):
        with (
            tc.tile_pool(name="persist", bufs=1) as persist,
            tc.tile_pool(name="consts", bufs=1) as consts,
        ):
            # Persistent SBUF tensors
            q_sb = persist.tile([128, FCH, S], BF16, tag="q_sb")
            k_sb = persist.tile([128, FCH, S], BF16, tag="k_sb")
            # V per head along dim2, each with a 65th ones column
            v_sb = persist.tile([128, NKB, HEADS_PER_CORE, HD + 1], BF16,
                                tag="v_sb")
            ot_sb = persist.tile([128, FCH, S], BF16, tag="ot_sb")
            vf_sb = persist.tile([128, FCH, S], BF16, tag="vf_sb")
            xt_sb = persist.tile([128, DCH, S], BF16, tag="xt_sb")

            cos_sb = consts.tile([128, S], BF16, tag="cos_sb")
            sin_sb = consts.tile([128, S], BF16, tag="sin_sb")
            mask_sb = consts.tile([128, 2, 128], BF16, tag="mask_sb")
            bd1_sb = consts.tile([128, 2], BF16, tag="bd1_sb")
            qbd_sb = consts.tile([2, 128], BF16, tag="qbd_sb")
            kbd_sb = consts.tile([2, 128], BF16, tag="kbd_sb")
            rotm_sb = consts.tile([128, 128], BF16, tag="rotm_sb")
            sinkf_sb = consts.tile([1, HEADS_PER_CORE], F32, tag="sinkf_sb")
            eps_sb = consts.tile([128, 1], F32, tag="eps_sb")
            ident_sb = consts.tile([128, 128], BF16, tag="ident_sb")
            mbias_sb = consts.tile([128, 128], BF16, tag="mbias_sb")
            if MASK_VIA_PE:
                nc.gpsimd.dma_start(ident_sb[:], identm)
                nc.gpsimd.dma_start(mbias_sb[:], maskbias)

            # weights
            wq_sb = persist.tile([128, DCH, FEATS], BF16, tag="wq_sb")
            wk_sb = persist.tile([128, DCH, FEATS], BF16, tag="wk_sb")
            wv_sb = persist.tile([128, DCH, FEATS], BF16, tag="wv_sb")
            wo_sb = persist.tile([128, FCH, D], BF16, tag="wo_sb")

            # --- input DMAs ------------------------------------------------
            # x in 16 [128,1024] halves (2KB partition lines = full DMA line
            # rate); the first-need set (all d of half 0 -> chunks s0+s1)
            # spread over the three DMA rings.  Weights are single
            # partition-major full-rate DMAs.
            def xh(eng, d, h):
                eng.dma_start(xt_sb[:, d, h * 1024:(h + 1) * 1024],
                              xT3[:, d, h * 1024:(h + 1) * 1024])

            nc.sync.dma_start(wq_sb[:], wqT3)
            xh(nc.sync, 0, 0), xh(nc.sync, 1, 0), xh(nc.sync, 2, 0)
            nc.sync.dma_start(bd1_sb[:], bd1)
            nc.sync.dma_start(qbd_sb[:], qbd)
            nc.sync.dma_start(kbd_sb[:], kbd)
            nc.sync.dma_start(rotm_sb[:], rotm)
            xh(nc.sync, 0, 1), xh(nc.sync, 1, 1), xh(nc.sync, 2, 1)

            xh(nc.scalar, 3, 0), xh(nc.scalar, 4, 0), xh(nc.scalar, 5, 0)
            nc.scalar.dma_start(cos_sb[:, 0:1024], cosT[:, 0:1024])
            nc.scalar.dma_start(sin_sb[:, 0:1024], sinT[:, 0:1024])
            xh(nc.scalar, 3, 1), xh(nc.scalar, 4, 1), xh(nc.scalar, 5, 1)
            nc.scalar.dma_start(cos_sb[:, 1024:S], cosT[:, 1024:S])
            nc.scalar.dma_start(sin_sb[:, 1024:S], sinT[:, 1024:S])

            # Warm-up, interleaved with the remaining loads: junk matmuls
            # open the HAM clock gate during the DMA ramp, a dummy
            # partition_broadcast forces the ~7us gpsimd LOAD_LIB early, and
            # dummy activations preload the ACT tables.  All tiles live in
            # the long-lived consts pool: a short-lived pool's exit barrier
            # would handcuff the vector queue to the slow gpsimd chain.
            wz = consts.tile([128, 256], BF16, tag="wz")
            nc.vector.memset(wz[:], 0.0)
            nc.vector.memset(eps_sb[:], EPS)
            nc.vector.memset(v_sb[:, :, :, HD:HD + 1], 1.0)
            wact = consts.tile([2, 16], F32, tag="wact")
            nc.vector.memset(wact[:], 1.0)

            xh(nc.gpsimd, 6, 0), xh(nc.gpsimd, 7, 0)
            nc.gpsimd.dma_start(wk_sb[:], wkT3)
            xh(nc.gpsimd, 6, 1), xh(nc.gpsimd, 7, 1)
            wbc = consts.tile([2, 16], F32, tag="wbc")
            nc.gpsimd.partition_broadcast(wbc[:], wact[0:1, :])
            nc.gpsimd.dma_start(wv_sb[:], wvT3)
            nc.gpsimd.dma_start(wo_sb[:], woT3)
            nc.gpsimd.dma_start(
                mask_sb[:], trimask2.rearrange("p (a b) -> p a b", a=2))
            nc.gpsimd.dma_start(sinkf_sb[:], sinkexp)

            wacto = consts.tile([2, 16], F32, tag="wacto")
            nc.scalar.activation(wacto[:], wact[:], EXP)
            nc.scalar.activation(wacto[:], wact[:], ARSQRT)
            nc.scalar.activation(wacto[:], wact[:], SQUARE)

            # ---------------- Phase A: Q/K projections + norm + rope ------
            # Two-stage software pipeline so the PE stream (proj of chunk c,
            # then sum-sq of c-1, then bsc/rot of c-2) never waits on the
            # scalar/vector chain of the chunk in flight.
            with (
                tc.tile_pool(name="pa_w", bufs=3) as paw,
                tc.tile_pool(name="pa_ps", bufs=2, space="PSUM") as paps,
                tc.tile_pool(name="pa_ss", bufs=2, space="PSUM") as passs,
                tc.tile_pool(name="pa_bsc", bufs=2, space="PSUM") as pabsc,
                tc.tile_pool(name="pa_rot", bufs=2, space="PSUM") as parot,
            ):
                # s-major so 4 consecutive chunks share one x quarter-set
                chunks = [(p, f, s) for s in range(NSQ)
                          for f in range(FCH) for p in ("q", "k")]

                # HAM warm-up junk matmuls during the input DMA ramp
                wps = paps.tile([128, SQ], F32, tag="qk_ps", name="wps")
                for i in range(36):
                    nc.tensor.matmul(wps[:, 0:256], wz[:, 0:128], wz[:],
                                     start=True, stop=True)

                def stage1(st):
                    p, f, s, ps = st
                    sq = paw.tile([128, SQ], BF16, tag="sq")
                    nc.scalar.square(sq[:], ps[:])
                    raw = paw.tile([128, SQ], BF16, tag="raw")
                    nc.scalar.copy(raw[:], ps[:])
                    ss_ps = passs.tile([128, SQ], F32, tag="ss_ps",
                                       name="ss_ps")[0:2]
                    nc.tensor.matmul(ss_ps[:], bd1_sb[:], sq[:],
                                     start=True, stop=True)
                    rinv = paw.tile([2, SQ], BF16, tag="rinv")
                    nc.scalar.activation(rinv[:], ss_ps[:], ARSQRT,
                                         bias=eps_sb[0:2, :], scale=1.0 / HD)
                    st.append(raw)
                    st.append(rinv)

                def stage2(st):
                    p, f, s, ps, raw, rinv = st
                    bdw_sb = qbd_sb if p == "q" else kbd_sb
                    dst_sb = q_sb if p == "q" else k_sb
                    cols = slice(s * SQ, (s + 1) * SQ)
                    bsc_ps = pabsc.tile([128, SQ], F32, tag="bsc_ps")
                    nc.tensor.matmul(bsc_ps[:], bdw_sb[:], rinv[:],
                                     start=True, stop=True)
                    qs = paw.tile([128, SQ], BF16, tag="qs")
                    nc.vector.tensor_mul(qs[:], raw[:], sin_sb[:, cols])
                    rot_ps = parot.tile([128, SQ], F32, tag="rot_ps")
                    nc.tensor.matmul(rot_ps[:], rotm_sb[:], qs[:],
                                     start=True, stop=True)
                    qc = paw.tile([128, SQ], BF16, tag="qc")
                    nc.vector.tensor_mul(qc[:], raw[:], cos_sb[:, cols])
                    qr = paw.tile([128, SQ], BF16, tag="qr")
                    nc.vector.tensor_add(qr[:], rot_ps[:], qc[:])
                    nc.vector.tensor_mul(dst_sb[:, f, cols], qr[:], bsc_ps[:])

                states = {}
                n = len(chunks)
                for c in range(n + 2):
                    if c < n:
                        p, f, s = chunks[c]
                        w_sb = wq_sb if p == "q" else wk_sb
                        cols = slice(s * SQ, (s + 1) * SQ)
                        ps = paps.tile([128, SQ], F32, tag="qk_ps")
                        for d in range(DCH):
                            nc.tensor.matmul(
                                ps[:],
                                w_sb[:, d, f * 128:(f + 1) * 128],
                                xt_sb[:, d, cols],
                                start=(d == 0),
                                stop=(d == DCH - 1),
                            )
                        if c < 4:
                            # early chunks are DMA-arrival-paced; pad the PE
                            # stream so the HAM clock gate stays open
                            for i in range(6):
                                nc.tensor.matmul(wps[:, 0:256], wz[:, 0:128],
                                                 wz[:], start=True, stop=True)
                        states[c] = [p, f, s, ps]
                    if c - 1 >= 0 and c - 1 < n:
                        stage1(states[c - 1])
                    if c - 2 >= 0:
                        stage2(states.pop(c - 2))
                # reload the EXP table set (evicted by SQUARE/ARSQRT) during
                # the phase transition instead of before the first real exp
                nc.scalar.activation(wacto[:], wact[:], EXP)

            # ------------- Phase C: attention + V-proj + output proj ------
            with (
                tc.tile_pool(name="pc_p", bufs=6) as pcp,
                tc.tile_pool(name="pc_w", bufs=2) as pcw,
                tc.tile_pool(name="pc_y", bufs=3) as pcy,
                tc.tile_pool(name="pc_s_ps", bufs=2, space="PSUM") as pcsps,
                tc.tile_pool(name="pc_o_ps", bufs=1, space="PSUM") as pcops,
                tc.tile_pool(name="pc_y_ps", bufs=2, space="PSUM") as pcyps,
            ):
                def v_proj(qt):
                    # V projection for one q-tile, borrowing the y PSUM pool
                    v_ps = pcyps.tile([128, SQ], F32, tag="y_ps", name="v_ps")
                    for d in range(DCH):
                        nc.tensor.matmul(
                            v_ps[:, 0:FEATS],
                            xt_sb[:, d, qt * 128:(qt + 1) * 128],
                            wv_sb[:, d, :],
                            start=(d == 0),
                            stop=(d == DCH - 1),
                        )
                    nc.vector.tensor_copy(
                        v_sb[:, qt, :, 0:HD],
                        v_ps[:, 0:FEATS].rearrange("p (h e) -> p h e",
                                                   h=HEADS_PER_CORE))

                vpend, ypend = [], []

                def mk_v_halves(qt):
                    # V projection split into two 4-matmul halves so a
                    # filler never exceeds the exp pipeline's ~1-iter
                    # run-ahead
                    st = {}

                    def h0():
                        st["ps"] = pcyps.tile([128, SQ], F32, tag="y_ps",
                                              name="v_ps")
                        for d in range(4):
                            nc.tensor.matmul(
                                st["ps"][:, 0:FEATS],
                                xt_sb[:, d, qt * 128:(qt + 1) * 128],
                                wv_sb[:, d, :],
                                start=(d == 0), stop=False)

                    def h1():
                        for d in range(4, DCH):
                            nc.tensor.matmul(
                                st["ps"][:, 0:FEATS],
                                xt_sb[:, d, qt * 128:(qt + 1) * 128],
                                wv_sb[:, d, :],
                                start=False, stop=(d == DCH - 1))
                        nc.vector.tensor_copy(
                            v_sb[:, qt, :, 0:HD],
                            st["ps"][:, 0:FEATS].rearrange(
                                "p (h e) -> p h e", h=HEADS_PER_CORE))
                    return [h0, h1]

                def mk_out_halves(qt):
                    st = {}

                    def half(n):
                        if n == 0:
                            st["y"] = pcy.tile([128, D], F16, tag="y_sb",
                                               name="y_sb")
                        y_ps = pcyps.tile([128, SQ], F32, tag="y_ps",
                                          name="y_ps")
                        for c in range(FCH):
                            nc.tensor.matmul(
                                y_ps[:],
                                ot_sb[:, c, qt * 128:(qt + 1) * 128],
                                wo_sb[:, c, n * SQ:(n + 1) * SQ],
                                start=(c == 0), stop=(c == FCH - 1))
                        nc.vector.tensor_copy(
                            st["y"][:, n * SQ:(n + 1) * SQ], y_ps[:])
                        if n == 1:
                            nc.sync.dma_start(y[qt * 128:(qt + 1) * 128, :],
                                              st["y"][:])
                    return [lambda: half(0), lambda: half(1)]

                def out_proj_qt(qt):
                    y_sb = pcy.tile([128, D], F16, tag="y_sb", name="y_sb")
                    for n in range(D // SQ):
                        y_ps = pcyps.tile([128, SQ], F32, tag="y_ps",
                                          name="y_ps")
                        for c in range(FCH):
                            nc.tensor.matmul(
                                y_ps[:],
                                ot_sb[:, c, qt * 128:(qt + 1) * 128],
                                wo_sb[:, c, n * SQ:(n + 1) * SQ],
                                start=(c == 0),
                                stop=(c == FCH - 1),
                            )
                        nc.vector.tensor_copy(y_sb[:, n * SQ:(n + 1) * SQ],
                                              y_ps[:])
                    nc.sync.dma_start(y[qt * 128:(qt + 1) * 128, :], y_sb[:])

                def attention_tf(t, f):
                    nkb = (t + 1) * (SQ // 128)
                    ot_ps = None
                    for i, kb in enumerate(range(nkb)):
                        j = kb - (t * (SQ // 128))
                        qlo = max(j, 0) * 128
                        diag = j >= 0
                        sp = pcsps.tile([128, 2, SQ], F32, tag="s_ps")
                        for hh in range(2):
                            plo = hh * HD
                            nc.tensor.matmul(
                                sp[:, hh, qlo:],
                                k_sb[plo:plo + HD, f, kb * 128:(kb + 1) * 128],
                                q_sb[plo:plo + HD, f, t * SQ + qlo:(t + 1) * SQ],
                                start=True, stop=True,
                            )
                        p_sb = pcp.tile([128, 2, SQ], BF16, tag="p_sb")
                        nc.scalar.activation(p_sb[:, :, qlo:],
                                             sp[:, :, qlo:], EXP)
                        if diag:
                            nc.vector.tensor_mul(
                                p_sb[:, :, qlo:qlo + 128],
                                p_sb[:, :, qlo:qlo + 128],
                                mask_sb[:])
                        if i == 0:
                            # allocated late: the single-buffered pool reuses
                            # the previous tile's banks only after its
                            # denominator reads drained
                            ot_ps = pcops.tile([HD + 1, 2, SQ], F32,
                                               tag="ot_ps", name="ot_ps")
                        for hh in range(2):
                            h = 2 * f + hh
                            nc.tensor.matmul(
                                ot_ps[:, hh, qlo:],
                                v_sb[:, kb, h, :],
                                p_sb[:, hh, qlo:],
                                start=(i == 0),
                                stop=(i == nkb - 1),
                            )
                        # half-size V/out-proj fillers absorb the PE idle
                        # that the exp-paced pipeline leaves each iteration
                        if i % 2 == 1:
                            if vpend:
                                vpend.pop(0)()
                            elif i % 4 == 3 and ypend:
                                ypend.pop(0)()
                    # denominators: row HD of ot_ps + exp(sink), batched hh
                    draw = pcw.tile([1, 2, SQ], F32, tag="draw")
                    nc.vector.tensor_scalar_add(
                        draw[:, 0, :], ot_ps[HD:HD + 1, 0, :],
                        sinkf_sb[0:1, 2 * f:2 * f + 1])
                    nc.vector.tensor_scalar_add(
                        draw[:, 1, :], ot_ps[HD:HD + 1, 1, :],
                        sinkf_sb[0:1, 2 * f + 1:2 * f + 2])
                    den = pcw.tile([1, 2, SQ], F32, tag="den")
                    nc.vector.reciprocal_approx_fast(den[:], draw[:])
                    last = (t == NSQ - 1 and f == 1)
                    if not last:
                        # drain ot_ps via the boundary-idle scalar engine so
                        # the single-buffered pool frees before the next
                        # tile's first PV matmul
                        ot_u = pcw.tile([HD, 2, SQ], BF16, tag="ot_u")
                        nc.scalar.copy(ot_u[:], ot_ps[0:HD, :, :])
                    for hh in range(2):
                        plo = hh * HD
                        bc_sb = pcw.tile([HD, SQ], F32, tag=f"bc_sb{hh}")
                        nc.gpsimd.partition_broadcast(bc_sb[:], den[:, hh, :])
                        nc.vector.tensor_mul(
                            ot_sb[plo:plo + HD, f, t * SQ:(t + 1) * SQ],
                            ot_ps[0:HD, hh, :] if last else ot_u[:, hh, :],
                            bc_sb[:])
                    if f == 1:
                        for qi in range(SQ // 128):
                            qt = t * (SQ // 128) + qi
                            ypend.extend(mk_out_halves(qt))

                # V q-tiles feed attention tile t's key blocks (<= 4t+3),
                # so they are threaded through the schedule just ahead of
                # need; out_proj q-tiles trail as PE fillers inside later
                # attention kb loops.
                junk_ps0 = pcsps.tile([128, 2, SQ], F32, tag="s_ps",
                                      name="junk_ps0")
                for i in range(8):
                    nc.tensor.matmul(junk_ps0[:, 0, 0:256], wz[:, 0:128],
                                     wz[:], start=True, stop=True)
                def flushv():
                    while vpend:
                        vpend.pop(0)()

                for qt in range(4):
                    v_proj(qt)
                vpend.extend(mk_v_halves(4) + mk_v_halves(5))
                attention_tf(0, 0)
                flushv()
                vpend.extend(mk_v_halves(6) + mk_v_halves(7))
                attention_tf(0, 1)
                flushv()
                vpend.extend(mk_v_halves(8) + mk_v_halves(9))
                attention_tf(1, 0)
                flushv()
                vpend.extend(mk_v_halves(10) + mk_v_halves(11))
                attention_tf(1, 1)
                flushv()
                while len(ypend) > 4:
                    ypend.pop(0)()
                vpend.extend(mk_v_halves(12) + mk_v_halves(13))
                attention_tf(2, 0)
                flushv()
                vpend.extend(mk_v_halves(14) + mk_v_halves(15))
                attention_tf(2, 1)
                flushv()
                while len(ypend) > 4:
                    ypend.pop(0)()
                attention_tf(3, 0)
                attention_tf(3, 1)
                junk_ps = pcsps.tile([128, 2, SQ], F32, tag="s_ps",
                                     name="junk_ps")

                def junk(n):
                    # keep the HAM clock gate open across dependency waits
                    for i in range(n):
                        nc.tensor.matmul(junk_ps[:, 0, 0:256], wz[:, 0:128],
                                         wz[:], start=True, stop=True)
                junk(6)
                while ypend:
                    ypend.pop(0)()
                    junk(2)

    nc.compile()
    return nc


_NC_CACHE = None


def _get_program():
    global _NC_CACHE
    if _NC_CACHE is None:
        _NC_CACHE = build_program()
    return _NC_CACHE


def _b(x):
    return np.ascontiguousarray(np.asarray(x, dtype=np.float32)).astype(BF16_NP)


def _shuf(wT, n):
    """[O*128, n] -> partition-major [128, O*n] (one full-line-rate DMA)."""
    o = wT.shape[0] // 128
    return np.ascontiguousarray(
        wT.reshape(o, 128, n).transpose(1, 0, 2).reshape(128, o * n))


def _host_inputs(x, wq, wk, wv, wo, q_norm_w, k_norm_w, sink_logit):
    """Build the 8 per-core input maps."""
    x = np.asarray(x, dtype=np.float32)
    wq = np.asarray(wq, dtype=np.float32)
    wk = np.asarray(wk, dtype=np.float32)
    wv = np.asarray(wv, dtype=np.float32)
    wo = np.asarray(wo, dtype=np.float32)
    q_norm_w = np.asarray(q_norm_w, dtype=np.float32)
    k_norm_w = np.asarray(k_norm_w, dtype=np.float32)
    sink_logit = np.asarray(sink_logit, dtype=np.float32)

    # rope tables, feature-major, duplicated across the two heads per chunk
    inv_freq = 1.0 / (ROPE_BASE ** (np.arange(0, HD, 2, dtype=np.float32) / HD))
    tpos = np.arange(S, dtype=np.float32)
    freqs = tpos[:, None] * inv_freq[None, :]           # [S, 32]
    emb = np.concatenate([freqs, freqs], axis=-1)       # [S, 64]
    cosT = _b(np.tile(np.cos(emb).T, (2, 1)))           # [128, S]
    sinT = _b(np.tile(np.sin(emb).T, (2, 1)))

    # triangular causal mask for the diagonal 128-wide band, doubled for
    # the two heads of a chunk
    kk = np.arange(128)[:, None]
    qq = np.arange(128)[None, :]
    tm = (kk <= qq).astype(np.float32)                  # [128, 128]
    trimask2 = _b(np.concatenate([tm, tm], axis=1))     # [128, 256]

    bd1 = np.zeros((128, 2), dtype=np.float32)
    bd1[0:64, 0] = 1.0
    bd1[64:128, 1] = 1.0
    bd1 = _b(bd1)

    scale_half = float(HD) ** -0.25  # sqrt of softmax scale, folded into q & k
    qbd = np.zeros((2, 128), dtype=np.float32)
    kbd = np.zeros((2, 128), dtype=np.float32)
    for m in range(2):
        qbd[m, m * 64:(m + 1) * 64] = q_norm_w * scale_half
        kbd[m, m * 64:(m + 1) * 64] = k_norm_w * scale_half
    qbd = _b(qbd)
    kbd = _b(kbd)

    rotm = np.zeros((128, 128), dtype=np.float32)
    for base in (0, 64):
        for m in range(32):
            rotm[base + m + 32, base + m] = -1.0
            rotm[base + m, base + m + 32] = 1.0
    rotm = _b(rotm)

    identm = _b(np.eye(128, dtype=np.float32))
    maskbias = _b(np.where(kk > qq, -30.0, 0.0).astype(np.float32))

    in_maps = []
    xT_b = [_b(x[b].T) for b in range(B)]
    for core in range(N_CORES):
        b = core // 4
        hg = core % 4
        rows = slice(hg * FEATS, (hg + 1) * FEATS)
        heads = slice(hg * HEADS_PER_CORE, (hg + 1) * HEADS_PER_CORE)
        in_maps.append({
            "xT": xT_b[b],
            "wqS": _shuf(_b(wq[rows, :].T), FEATS),
            "wkS": _shuf(_b(wk[rows, :].T), FEATS),
            "wvS": _shuf(_b(wv[rows, :].T), FEATS),
            "woS": _shuf(_b(wo[:, rows].T), D),
            "cosT": cosT,
            "sinT": sinT,
            "trimask2": trimask2,
            "identm": identm,
            "maskbias": maskbias,
            "bd1": bd1,
            "qbd": qbd,
            "kbd": kbd,
            "rotm": rotm,
            "sinkexp": np.exp(sink_logit[heads]).astype(np.float32).reshape(
                1, HEADS_PER_CORE),
        })
    return in_maps


def kernel(x, wq, wk, wv, wo, q_norm_w, k_norm_w, sink_logit, _run_kwargs=None):
    nc = _get_program()
    in_maps = _host_inputs(x, wq, wk, wv, wo, q_norm_w, k_norm_w, sink_logit)
    res = run_bass_kernel_spmd(nc, in_maps, core_ids=list(range(N_CORES)),
                               **(_run_kwargs or {}))
    out = np.zeros((B, S, D), dtype=np.float32)
    for core in range(N_CORES):
        out[core // 4] += res.results[core]["y"].astype(np.float32)
    if _run_kwargs:
        kernel.last_result = res
    return out
